# revision 8
# baseline (speedup 1.0000x reference)
"""Inclusive prefix-sum (Blelloch scan, additive) along L for X_in (8, 4096, 64, 16) f32.

Sharding: batch B=8 across the 8 NeuronCores (one batch per core; no communication).
Per core: cumsum along L=4096 of an (L, F=1024) matrix. Input is bf16 (host casts
f32->bf16 with a 2^-5 scale folded in), output is fp8 e3m4 (4 mantissa bits);
the global 2^-5 scale keeps |y| <= ~12 < 15.5 (e3m4 max) and the host multiplies
the upcast result by 32 (exact). Near-zero elements land in e3m4 subnormals;
their norm contribution is negligible. Full-pipeline rel err ~1.35e-2 vs the
2e-2 gate (bf16-everywhere was 2.3e-3). HBM traffic: 8 MiB in + 4 MiB out per
core = 12 MiB vs 16 MiB all-bf16 -> DMA floor ~35 us at the ~358 GB/s per-core
HBM limit.

Per-core structure ("hybrid PE scan + fused combine"):
  - PE: per 4-block chunk and 128-feature group, four 128x128 bf16 matmuls with
    the data stationary and an upper-triangular ones matrix moving produce a
    [128(f) x 512(l)] f32 PSUM tile of transposed in-block inclusive scans.
  - Carries: per group a persistent ct[128, 33] f32 tile holds the carry after
    blocks 0..j-1. One FD=4 DVE tensor_tensor_scan over the PSUM block totals
    (ps[:, 127::128], f32 state) extends the chain 4 columns per chunk.
  - Combine: groups 0-4 on DVE as ONE fused FD=512 tensor_tensor add per chunk
    (carry operand = ct columns broadcast via a stride-0 inner AP dim); groups
    5-7 on ACT as per-block scalar.add with the ct column as bias. Outputs are
    staged bf16 and DMA'd to y (F, L); numpy un-transposes when unsharding.
  - DMA: input chunks (1 MiB, 4 L-blocks) are hoisted and issued on the (otherwise
    idle) GPSIMD SWDGE ring so they never queue behind compute-dependent output DMAs; outputs are staged double-width
    (2 chunks -> 32x 256 KiB DMAs, 2 KiB per-partition runs) on the sync ring;
    halving the out-DMA count measured ~12 us faster than 64x 128 KiB. Interleaving both on the sync
    ring costs ~4 us/iter (FIFO blocking), fully hoisting on one ring ~9 us.

Measured (For_i loop-diff on HW, 8 cores concurrent): 52532 ns/iter incl. the
~13.5 us For_i back-edge (devloop harness; full test.py runs measure 47-56k
depending on device state). f32 baseline: 109-113k ns/iter.
"""

import numpy as np

B, L, D, N = 8, 4096, 64, 16
F = D * N
NCORES = 8
LBLK = 128
NGROUP = F // 128     # 8
NBLK = L // LBLK      # 32
CHUNK = 4             # L-blocks per input DMA chunk == blocks per psum tile
NCHUNK = NBLK // CHUNK  # 8
PSPAN = CHUNK * LBLK  # 512 L-cols per psum tile
NDVE = 5              # groups on DVE fused path; rest on ACT
XIN_BUFS = 8
SPAIR = 4             # chunks staged per output DMA (4 -> 2 KiB fp8 runs)
ISCALE = 2.0 ** -5    # folded into host bf16 cast; keeps |y| < 15.5 (e3m4 max)
OSCALE = 32.0

_CACHE = {}


def _build_nc(loop_nrep=None):
    from contextlib import nullcontext

    import concourse.bacc as bacc
    import concourse.mybir as mybir
    from concourse.tile import TileContext

    f32 = mybir.dt.float32
    bf16 = mybir.dt.bfloat16
    fp8 = mybir.dt.float8e3
    add = mybir.AluOpType.add
    bypass = mybir.AluOpType.bypass
    nc = bacc.Bacc(
        "TRN2", target_bir_lowering=False, debug=False, num_devices=NCORES
    )
    x = nc.dram_tensor("x", (L, F), bf16, kind="ExternalInput")
    u = nc.dram_tensor("u", (LBLK, LBLK), bf16, kind="ExternalInput")
    y = nc.dram_tensor("y", (F, L), fp8, kind="ExternalOutput")

    with TileContext(nc) as tc:
        with (
            tc.tile_pool(name="const", bufs=1) as cpool,
            tc.tile_pool(name="xin", bufs=XIN_BUFS) as xpool,
            tc.tile_pool(name="stage", bufs=2) as spool,
            tc.tile_pool(name="psum", bufs=8, space="PSUM") as ppool,
        ):
            ut = cpool.tile([LBLK, LBLK], bf16)
            nc.sync.dma_start(out=ut[:], in_=u[:, :])
            cts = []
            for g in range(NGROUP):
                ct = cpool.tile([128, NBLK + 1], f32, name=f"ct{g}")
                nc.vector.memset(ct[:, 0:1], 0.0)
                cts.append(ct)
            dz = cpool.tile([128, CHUNK], f32)
            nc.vector.memset(dz[:], 0.0)

            loop_cm = tc.For_i(0, loop_nrep, 1) if loop_nrep else nullcontext()
            loop_cm.__enter__()
            xts = []
            stw = [None] * NGROUP
            for ii in range(NCHUNK):
                xt = xpool.tile([128, CHUNK * F], bf16, tag="xt", name=f"xt_{ii}")
                nc.gpsimd.dma_start(
                    out=xt[:],
                    in_=x[ii * PSPAN : (ii + 1) * PSPAN, :].rearrange(
                        "(t p) f -> p t f", p=128
                    ),
                )
                xts.append(xt)
            for ii in range(NCHUNK):
                xt = xts[ii]
                i0 = ii * CHUNK
                for g in range(NGROUP):
                    ct = cts[g]
                    ps = ppool.tile([128, PSPAN], f32, tag="ps", name=f"ps_{ii}_{g}")
                    for j in range(CHUNK):
                        nc.tensor.matmul(
                            ps[:, j * LBLK : (j + 1) * LBLK],
                            xt[:, j * F + g * LBLK : j * F + (g + 1) * LBLK],
                            ut[:],
                            start=True,
                            stop=True,
                        )
                    # extend carry chain by 4: ct[:, i0+1 : i0+5]
                    nc.vector.tensor_tensor_scan(
                        out=ct[:, i0 + 1 : i0 + 1 + CHUNK],
                        data0=ps[:, LBLK - 1 : PSPAN : LBLK],
                        data1=dz[:],
                        initial=ct[:, i0 : i0 + 1],
                        op0=add,
                        op1=bypass,
                    )
                    if ii % SPAIR == 0:
                        stw[g] = spool.tile(
                            [128, SPAIR * PSPAN], fp8, tag=f"st{g}", name=f"st{g}_{ii}"
                        )
                    st = stw[g][:, (ii % SPAIR) * PSPAN : (ii % SPAIR + 1) * PSPAN]
                    if g < NDVE:
                        nc.vector.tensor_tensor(
                            out=st.rearrange("p (c l) -> p c l", c=CHUNK),
                            in0=ps[:].rearrange("p (c l) -> p c l", c=CHUNK),
                            in1=ct[:, i0 : i0 + CHUNK].rearrange(
                                "p (c o) -> p c o", o=1
                            ).broadcast_to([128, CHUNK, LBLK]),
                            op=add,
                        )
                    else:
                        for j in range(CHUNK):
                            nc.scalar.add(
                                out=st[:, j * LBLK : (j + 1) * LBLK],
                                in_=ps[:, j * LBLK : (j + 1) * LBLK],
                                add=ct[:, i0 + j : i0 + j + 1],
                            )
                    if ii % SPAIR == SPAIR - 1:
                        nc.sync.dma_start(
                            out=y[
                                g * LBLK : (g + 1) * LBLK,
                                (ii - (SPAIR - 1)) * PSPAN : (ii + 1) * PSPAN,
                            ],
                            in_=stw[g][:],
                        )
            loop_cm.__exit__(None, None, None)
    nc.compile()
    return nc


def _get_nc():
    if "nc" not in _CACHE:
        _CACHE["nc"] = _build_nc()
    return _CACHE["nc"]


def _make_in_maps(X_in):
    import ml_dtypes

    xs = np.ascontiguousarray(
        (np.asarray(X_in, dtype=np.float32) * np.float32(ISCALE)).astype(
            ml_dtypes.bfloat16
        )
    ).reshape(B, L, F)
    umat = np.triu(np.ones((LBLK, LBLK), dtype=np.float32)).astype(ml_dtypes.bfloat16)
    return [{"x": xs[b], "u": umat} for b in range(B)]


def _unshard(per_core_outs):
    out = np.empty((B, L, D, N), dtype=np.float32)
    for b in range(B):
        yb = per_core_outs[b]["y"].astype(np.float32) * np.float32(OSCALE)
        out[b] = yb.T.reshape(L, D, N)
    return out


def kernel(X_in):
    from concourse.bass_utils import run_bass_kernel_spmd

    nc = _get_nc()
    res = run_bass_kernel_spmd(nc, _make_in_maps(X_in), core_ids=list(range(NCORES)))
    return _unshard(res.results)



# revision 12
# speedup vs baseline: 1.0008x; 1.0008x over previous
"""Inclusive prefix-sum (Blelloch scan, additive) along L for X_in (8, 4096, 64, 16) f32.

Sharding: batch B=8 across the 8 NeuronCores (one batch per core; no communication).
Per core: cumsum along L=4096 of an (L, F=1024) matrix. Input is bf16 (host casts
f32->bf16 with a 2^-5 scale folded in), output is fp8 e3m4 (4 mantissa bits);
the global 2^-5 scale keeps |y| <= ~12 < 15.5 (e3m4 max) and the host multiplies
the upcast result by 32 (exact). Near-zero elements land in e3m4 subnormals;
their norm contribution is negligible. Full-pipeline rel err ~1.35e-2 vs the
2e-2 gate (bf16-everywhere was 2.3e-3). HBM traffic: 8 MiB in + 4 MiB out per
core = 12 MiB vs 16 MiB all-bf16 -> DMA floor ~35 us at the ~358 GB/s per-core
HBM limit.

Per-core structure ("hybrid PE scan + fused combine"):
  - PE: per 4-block chunk and 128-feature group, four 128x128 bf16 matmuls with
    the data stationary and an upper-triangular ones matrix moving produce a
    [128(f) x 512(l)] f32 PSUM tile of transposed in-block inclusive scans.
  - Carries: per group a persistent ct[128, 33] f32 tile holds the carry after
    blocks 0..j-1. One FD=4 DVE tensor_tensor_scan over the PSUM block totals
    (ps[:, 127::128], f32 state) extends the chain 4 columns per chunk.
  - Combine: groups 0-4 on DVE as ONE fused FD=512 tensor_tensor add per chunk
    (carry operand = ct columns broadcast via a stride-0 inner AP dim); groups
    5-7 on ACT as per-block scalar.add with the ct column as bias. Outputs are
    staged bf16 and DMA'd to y (F, L); numpy un-transposes when unsharding.
  - DMA: input chunks (1 MiB, 4 L-blocks) are hoisted and issued on the (otherwise
    idle) GPSIMD SWDGE ring so they never queue behind compute-dependent output DMAs; outputs are staged double-width
    (2 chunks -> 32x 256 KiB DMAs, 2 KiB per-partition runs) on the sync ring;
    halving the out-DMA count measured ~12 us faster than 64x 128 KiB. Interleaving both on the sync
    ring costs ~4 us/iter (FIFO blocking), fully hoisting on one ring ~9 us.

Measured (For_i loop-diff on HW, 8 cores concurrent): 52532 ns/iter incl. the
~13.5 us For_i back-edge (devloop harness; full test.py runs measure 47-56k
depending on device state). f32 baseline: 109-113k ns/iter.
"""

import numpy as np

B, L, D, N = 8, 4096, 64, 16
F = D * N
NCORES = 8
LBLK = 128
NGROUP = F // 128     # 8
NBLK = L // LBLK      # 32
CHUNK = 4             # L-blocks per input DMA chunk == blocks per psum tile
NCHUNK = NBLK // CHUNK  # 8
PSPAN = CHUNK * LBLK  # 512 L-cols per psum tile
NDVE = 5              # groups on DVE fused path; rest on ACT
XIN_BUFS = 8
SPAIR = 4             # chunks staged per output DMA (4 -> 2 KiB fp8 runs)
ISCALE = 2.0 ** -5    # folded into host bf16 cast; keeps |y| < 15.5 (e3m4 max)
OSCALE = 32.0

_CACHE = {}


def _build_nc(loop_nrep=None):
    from contextlib import nullcontext

    import concourse.bacc as bacc
    import concourse.mybir as mybir
    from concourse.tile import TileContext

    f32 = mybir.dt.float32
    bf16 = mybir.dt.bfloat16
    fp8 = mybir.dt.float8e3
    add = mybir.AluOpType.add
    bypass = mybir.AluOpType.bypass
    nc = bacc.Bacc(
        "TRN2", target_bir_lowering=False, debug=False, num_devices=NCORES
    )
    x = nc.dram_tensor("x", (L, F), bf16, kind="ExternalInput")
    u = nc.dram_tensor("u", (LBLK, LBLK), bf16, kind="ExternalInput")
    y = nc.dram_tensor("y", (F, L), fp8, kind="ExternalOutput")

    with TileContext(nc) as tc:
        with (
            tc.tile_pool(name="const", bufs=1) as cpool,
            tc.tile_pool(name="xin", bufs=XIN_BUFS) as xpool,
            tc.tile_pool(name="stage", bufs=2) as spool,
            tc.tile_pool(name="psum", bufs=8, space="PSUM") as ppool,
        ):
            ut = cpool.tile([LBLK, LBLK], bf16)
            nc.sync.dma_start(out=ut[:], in_=u[:, :])
            cts = []
            for g in range(NGROUP):
                ct = cpool.tile([128, NBLK + 1], f32, name=f"ct{g}")
                nc.vector.memset(ct[:, 0:1], 0.0)
                cts.append(ct)
            dz = cpool.tile([128, CHUNK], f32)
            nc.vector.memset(dz[:], 0.0)

            loop_cm = tc.For_i(0, loop_nrep, 1) if loop_nrep else nullcontext()
            loop_cm.__enter__()
            xts = []
            stw = [None] * NGROUP
            for ii in range(NCHUNK):
                xt = xpool.tile([128, CHUNK * F], bf16, tag="xt", name=f"xt_{ii}")
                eng = nc.gpsimd if ii % 2 == 0 else nc.sync
                eng.dma_start(
                    out=xt[:],
                    in_=x[ii * PSPAN : (ii + 1) * PSPAN, :].rearrange(
                        "(t p) f -> p t f", p=128
                    ),
                )
                xts.append(xt)
            for ii in range(NCHUNK):
                xt = xts[ii]
                i0 = ii * CHUNK
                for g in range(NGROUP):
                    ct = cts[g]
                    ps = ppool.tile([128, PSPAN], f32, tag="ps", name=f"ps_{ii}_{g}")
                    for j in range(CHUNK):
                        nc.tensor.matmul(
                            ps[:, j * LBLK : (j + 1) * LBLK],
                            xt[:, j * F + g * LBLK : j * F + (g + 1) * LBLK],
                            ut[:],
                            start=True,
                            stop=True,
                        )
                    # extend carry chain by 4: ct[:, i0+1 : i0+5]
                    nc.vector.tensor_tensor_scan(
                        out=ct[:, i0 + 1 : i0 + 1 + CHUNK],
                        data0=ps[:, LBLK - 1 : PSPAN : LBLK],
                        data1=dz[:],
                        initial=ct[:, i0 : i0 + 1],
                        op0=add,
                        op1=bypass,
                    )
                    if ii % SPAIR == 0:
                        stw[g] = spool.tile(
                            [128, SPAIR * PSPAN], fp8, tag=f"st{g}", name=f"st{g}_{ii}"
                        )
                    st = stw[g][:, (ii % SPAIR) * PSPAN : (ii % SPAIR + 1) * PSPAN]
                    if g < NDVE:
                        nc.vector.tensor_tensor(
                            out=st.rearrange("p (c l) -> p c l", c=CHUNK),
                            in0=ps[:].rearrange("p (c l) -> p c l", c=CHUNK),
                            in1=ct[:, i0 : i0 + CHUNK].rearrange(
                                "p (c o) -> p c o", o=1
                            ).broadcast_to([128, CHUNK, LBLK]),
                            op=add,
                        )
                    else:
                        for j in range(CHUNK):
                            nc.scalar.add(
                                out=st[:, j * LBLK : (j + 1) * LBLK],
                                in_=ps[:, j * LBLK : (j + 1) * LBLK],
                                add=ct[:, i0 + j : i0 + j + 1],
                            )
                    if ii % SPAIR == SPAIR - 1:
                        nc.scalar.dma_start(
                            out=y[
                                g * LBLK : (g + 1) * LBLK,
                                (ii - (SPAIR - 1)) * PSPAN : (ii + 1) * PSPAN,
                            ],
                            in_=stw[g][:],
                        )
            loop_cm.__exit__(None, None, None)
    nc.compile()
    return nc


def _get_nc():
    if "nc" not in _CACHE:
        _CACHE["nc"] = _build_nc()
    return _CACHE["nc"]


def _make_in_maps(X_in):
    import ml_dtypes

    xs = np.ascontiguousarray(
        (np.asarray(X_in, dtype=np.float32) * np.float32(ISCALE)).astype(
            ml_dtypes.bfloat16
        )
    ).reshape(B, L, F)
    umat = np.triu(np.ones((LBLK, LBLK), dtype=np.float32)).astype(ml_dtypes.bfloat16)
    return [{"x": xs[b], "u": umat} for b in range(B)]


def _unshard(per_core_outs):
    out = np.empty((B, L, D, N), dtype=np.float32)
    for b in range(B):
        yb = per_core_outs[b]["y"].astype(np.float32) * np.float32(OSCALE)
        out[b] = yb.T.reshape(L, D, N)
    return out


def kernel(X_in):
    from concourse.bass_utils import run_bass_kernel_spmd

    nc = _get_nc()
    res = run_bass_kernel_spmd(nc, _make_in_maps(X_in), core_ids=list(range(NCORES)))
    return _unshard(res.results)



# revision 13
# speedup vs baseline: 1.0504x; 1.0496x over previous
"""Inclusive prefix-sum (Blelloch scan, additive) along L for X_in (8, 4096, 64, 16) f32.

Sharding: batch B=8 across the 8 NeuronCores (one batch per core; no communication).
Per core: cumsum along L=4096 of an (L, F=1024) matrix. Input is bf16 (host casts
f32->bf16 with a 2^-5 scale folded in), output is fp8 e3m4 (4 mantissa bits);
the global 2^-5 scale keeps |y| <= ~12 < 15.5 (e3m4 max) and the host multiplies
the upcast result by 32 (exact). Near-zero elements land in e3m4 subnormals;
their norm contribution is negligible. Full-pipeline rel err ~1.35e-2 vs the
2e-2 gate (bf16-everywhere was 2.3e-3). HBM traffic: 8 MiB in + 4 MiB out per
core = 12 MiB vs 16 MiB all-bf16 -> DMA floor ~35 us at the ~358 GB/s per-core
HBM limit.

Per-core structure ("hybrid PE scan + fused combine"):
  - PE: per 4-block chunk and 128-feature group, four 128x128 bf16 matmuls with
    the data stationary and an upper-triangular ones matrix moving produce a
    [128(f) x 512(l)] f32 PSUM tile of transposed in-block inclusive scans.
  - Carries: per group a persistent ct[128, 33] f32 tile holds the carry after
    blocks 0..j-1. One FD=4 DVE tensor_tensor_scan over the PSUM block totals
    (ps[:, 127::128], f32 state) extends the chain 4 columns per chunk.
  - Combine: groups 0-4 on DVE as ONE fused FD=512 tensor_tensor add per chunk
    (carry operand = ct columns broadcast via a stride-0 inner AP dim); groups
    5-7 on ACT as per-block scalar.add with the ct column as bias. Outputs are
    staged bf16 and DMA'd to y (F, L); numpy un-transposes when unsharding.
  - DMA: input chunks (1 MiB, 4 L-blocks) are hoisted and issued on the (otherwise
    idle) GPSIMD SWDGE ring so they never queue behind compute-dependent output DMAs; outputs are staged double-width
    (2 chunks -> 32x 256 KiB DMAs, 2 KiB per-partition runs) on the sync ring;
    halving the out-DMA count measured ~12 us faster than 64x 128 KiB. Interleaving both on the sync
    ring costs ~4 us/iter (FIFO blocking), fully hoisting on one ring ~9 us.

Measured (For_i loop-diff on HW, 8 cores concurrent): 52532 ns/iter incl. the
~13.5 us For_i back-edge (devloop harness; full test.py runs measure 47-56k
depending on device state). f32 baseline: 109-113k ns/iter.
"""

import numpy as np

B, L, D, N = 8, 4096, 64, 16
F = D * N
NCORES = 8
LBLK = 128
NGROUP = F // 128     # 8
NBLK = L // LBLK      # 32
CHUNK = 4             # L-blocks per input DMA chunk == blocks per psum tile
NCHUNK = NBLK // CHUNK  # 8
PSPAN = CHUNK * LBLK  # 512 L-cols per psum tile
NDVE = 5              # groups on DVE fused path; rest on ACT
XIN_BUFS = 8
SPAIR = 4             # chunks staged per output DMA (4 -> 2 KiB fp8 runs)
ISCALE = 2.0 ** -5    # folded into host bf16 cast; keeps |y| < 15.5 (e3m4 max)
OSCALE = 32.0

_CACHE = {}


def _build_nc(loop_nrep=None):
    from contextlib import nullcontext

    import concourse.bacc as bacc
    import concourse.mybir as mybir
    from concourse.tile import TileContext

    f32 = mybir.dt.float32
    bf16 = mybir.dt.bfloat16
    fp8 = mybir.dt.float8e3
    add = mybir.AluOpType.add
    bypass = mybir.AluOpType.bypass
    nc = bacc.Bacc(
        "TRN2", target_bir_lowering=False, debug=False, num_devices=NCORES
    )
    x = nc.dram_tensor("x", (L, F), bf16, kind="ExternalInput")
    u = nc.dram_tensor("u", (LBLK, LBLK), bf16, kind="ExternalInput")
    y = nc.dram_tensor("y", (F, L), fp8, kind="ExternalOutput")

    with TileContext(nc) as tc:
        with (
            tc.tile_pool(name="const", bufs=1) as cpool,
            tc.tile_pool(name="xin", bufs=XIN_BUFS) as xpool,
            tc.tile_pool(name="stage", bufs=2) as spool,
            tc.tile_pool(name="psum", bufs=8, space="PSUM") as ppool,
        ):
            ut = cpool.tile([LBLK, LBLK], bf16)
            nc.sync.dma_start(out=ut[:], in_=u[:, :])
            cts = []
            for g in range(NGROUP):
                ct = cpool.tile([128, NBLK + 1], f32, name=f"ct{g}")
                nc.vector.memset(ct[:, 0:1], 0.0)
                cts.append(ct)
            dz = cpool.tile([128, CHUNK], f32)
            nc.vector.memset(dz[:], 0.0)

            loop_cm = (
                tc.For_i(0, loop_nrep, 1, staggered_reset=True)
                if loop_nrep
                else nullcontext()
            )
            loop_cm.__enter__()
            xts = []
            stw = [None] * NGROUP
            for ii in range(NCHUNK):
                xt = xpool.tile([128, CHUNK * F], bf16, tag="xt", name=f"xt_{ii}")
                eng = nc.gpsimd if ii % 2 == 0 else nc.sync
                eng.dma_start(
                    out=xt[:],
                    in_=x[ii * PSPAN : (ii + 1) * PSPAN, :].rearrange(
                        "(t p) f -> p t f", p=128
                    ),
                )
                xts.append(xt)
            for ii in range(NCHUNK):
                xt = xts[ii]
                i0 = ii * CHUNK
                for g in range(NGROUP):
                    ct = cts[g]
                    ps = ppool.tile([128, PSPAN], f32, tag="ps", name=f"ps_{ii}_{g}")
                    for j in range(CHUNK):
                        nc.tensor.matmul(
                            ps[:, j * LBLK : (j + 1) * LBLK],
                            xt[:, j * F + g * LBLK : j * F + (g + 1) * LBLK],
                            ut[:],
                            start=True,
                            stop=True,
                        )
                    # extend carry chain by 4: ct[:, i0+1 : i0+5]
                    nc.vector.tensor_tensor_scan(
                        out=ct[:, i0 + 1 : i0 + 1 + CHUNK],
                        data0=ps[:, LBLK - 1 : PSPAN : LBLK],
                        data1=dz[:],
                        initial=ct[:, i0 : i0 + 1],
                        op0=add,
                        op1=bypass,
                    )
                    if ii % SPAIR == 0:
                        stw[g] = spool.tile(
                            [128, SPAIR * PSPAN], fp8, tag=f"st{g}", name=f"st{g}_{ii}"
                        )
                    st = stw[g][:, (ii % SPAIR) * PSPAN : (ii % SPAIR + 1) * PSPAN]
                    if g < NDVE:
                        nc.vector.tensor_tensor(
                            out=st.rearrange("p (c l) -> p c l", c=CHUNK),
                            in0=ps[:].rearrange("p (c l) -> p c l", c=CHUNK),
                            in1=ct[:, i0 : i0 + CHUNK].rearrange(
                                "p (c o) -> p c o", o=1
                            ).broadcast_to([128, CHUNK, LBLK]),
                            op=add,
                        )
                    else:
                        for j in range(CHUNK):
                            nc.scalar.add(
                                out=st[:, j * LBLK : (j + 1) * LBLK],
                                in_=ps[:, j * LBLK : (j + 1) * LBLK],
                                add=ct[:, i0 + j : i0 + j + 1],
                            )
                    if ii % SPAIR == SPAIR - 1:
                        nc.scalar.dma_start(
                            out=y[
                                g * LBLK : (g + 1) * LBLK,
                                (ii - (SPAIR - 1)) * PSPAN : (ii + 1) * PSPAN,
                            ],
                            in_=stw[g][:],
                        )
            loop_cm.__exit__(None, None, None)
    nc.compile()
    return nc


def _get_nc():
    if "nc" not in _CACHE:
        _CACHE["nc"] = _build_nc()
    return _CACHE["nc"]


def _make_in_maps(X_in):
    import ml_dtypes

    xs = np.ascontiguousarray(
        (np.asarray(X_in, dtype=np.float32) * np.float32(ISCALE)).astype(
            ml_dtypes.bfloat16
        )
    ).reshape(B, L, F)
    umat = np.triu(np.ones((LBLK, LBLK), dtype=np.float32)).astype(ml_dtypes.bfloat16)
    return [{"x": xs[b], "u": umat} for b in range(B)]


def _unshard(per_core_outs):
    out = np.empty((B, L, D, N), dtype=np.float32)
    for b in range(B):
        yb = per_core_outs[b]["y"].astype(np.float32) * np.float32(OSCALE)
        out[b] = yb.T.reshape(L, D, N)
    return out


def kernel(X_in):
    from concourse.bass_utils import run_bass_kernel_spmd

    nc = _get_nc()
    res = run_bass_kernel_spmd(nc, _make_in_maps(X_in), core_ids=list(range(NCORES)))
    return _unshard(res.results)

